# revision 33
# baseline (speedup 1.0000x reference)
"""Bass/Trainium2 kernel for nn_BitGatConv (GAT-style message passing), V3.

Self-contained: takes full inputs, shards edges by destination window-pair
across 8 NeuronCores (SPMD, one program), returns the full [N, HC] output.

V3 vs V2 — driven by phase ablation (gather phase was +473 us of the 780 us
total, dominated by per-call SWDGE/HWDGE fixed overheads ~1-2.4 us x 392
calls) and the DVE perf-mode rules (broadcast operands force 1x mode):

  - Gathers are batched per GROUP of GP=7 destination pairs: 2 dma_gather
    calls per group (14 total vs 196), full-capacity idx streams (dummy
    row-0 pads, no -1 tails) so no G memset is needed.
  - All idx/aux streams are loaded in 4 up-front DMAs; phase A runs in
    8-tile chunks (1 read + 8 matmuls + 1 copy + 1 write per chunk).
  - One-hot builds avoid broadcast operands: OH (scatter one-hot,
    [slot,bin,tl]) is built with per-bin tensor_scalar is_equal against a
    materialized iota row (4x DVE mode, per-partition scalar = tl).
    OHT (transposed one-hot, [tl,bin,slot]) is unpacked from host-packed
    bitmasks with 16 dual-op tensor_scalar ops ((bits AND (1<<j)) != 0),
    also 4x; the bit layout matches slot s' via j = s'//8, w = s'%8.
  - s = att_i[tgt] + att_j[src] is formed ON PE: per bin, PSUM accumulates
    OHT_b.T @ ai_pair (att_i expand) + I.T @ G_aj (the gathered att_j),
    so DVE never touches fp32 PSUM. leaky relu + exp run on ACT (Lrelu
    reads PSUM natively, alpha=0.2; Exp writes back into G's aj half).
  - Per-pair DVE work is only: 18 OH builds + 16 OHT unpacks + y=h*x
    tensor_tensor (all 2-byte, stride-1, 2x/4x modes).

Algorithm per core (rotated node ids so all cores run the same program):
  Phase A: hj table rows [h | att_j] -> DRAM (lo/hi for int16 gather
    reach); ai_sb = nodes_ft @ (W@A1) for own shard -> SBUF.
  Phase B: per group: gather [h|aj] rows by src (full capacity); per pair:
    build OH/OHT; PE: ps_s = OHT_b.T@ai + aj; ACT: x = exp(lrelu(ps_s))
    -> G aj half; DVE: y = h*x -> G h half; PE: MM += OH_b.T @ [y|x].
  Phase C: out = numer / (denom + 1e-16) + bias.
  No segment-max subtraction: logits are bounded so exp is safe, and
  softmax is shift-free identical.
"""

import math
import os
import sys
from contextlib import ExitStack

import numpy as np

for _p in ("/opt/trn_rl_repo",):
    if _p not in sys.path:
        sys.path.insert(0, _p)

import ml_dtypes  # noqa: E402

BF16_NP = ml_dtypes.bfloat16

# ---------------------------------------------------------------------------
# Problem constants (hardcoded per contest rules)
N_NODES = 50000
N_EDGES = 800000
IN_CH = 128
HC = 64
NEG_SLOPE = 0.2
N_CORES = 8
PW = 128  # nodes per scatter pair (one-hot width)
GP = 4    # destination pairs per gather group


def _cfg(n_nodes=N_NODES, n_cores=N_CORES):
    npc = math.ceil(math.ceil(n_nodes / PW) / n_cores)  # pairs per core
    nshard = npc * PW
    n_pad = n_cores * nshard
    return dict(
        N=n_nodes, NC=n_cores, NPC=npc, NSHARD=nshard, N_PAD=n_pad,
        HL=n_pad // 2, T_TILES=n_pad // 128, NG=math.ceil(npc / GP),
    )


def _prep(inputs, cfg):
    """Host-side preprocessing: shard + pad + index building (numpy only)."""
    NC, NPC, NSHARD, N_PAD, HL = (
        cfg["NC"], cfg["NPC"], cfg["NSHARD"], cfg["N_PAD"], cfg["HL"])
    N = cfg["N"]

    nodes_ft = np.asarray(inputs["nodes_ft"], dtype=np.float32)
    adj = np.asarray(inputs["adj_list"])
    weight = np.asarray(inputs["weight"], dtype=np.float32)
    a1 = np.asarray(inputs["att_layer_1"], dtype=np.float32)
    a2 = np.asarray(inputs["att_layer_2"], dtype=np.float32)
    bias = np.asarray(inputs["bias"], dtype=np.float32)

    tgt = adj[0].astype(np.int64)
    src = adj[1].astype(np.int64)
    E = tgt.shape[0]

    pair = tgt // PW
    core = pair // NPC
    ploc = pair % NPC
    tl = (tgt % PW).astype(np.int64)

    src_rot = (src - core * NSHARD) % N_PAD
    half = (src_rot >= HL).astype(np.int64)
    idx16 = (src_rot - half * HL).astype(np.int16)

    grp = (core * NPC + ploc) * 2 + half
    cnt2 = np.bincount(grp, minlength=NC * NPC * 2)
    K_LO = max(1, int(math.ceil(cnt2[0::2].max() / 128.0)))
    K_HI = max(1, int(math.ceil(cnt2[1::2].max() / 128.0)))
    NB = K_LO + K_HI

    # sort each (core, pair, half) segment by source row: the gather's DMA
    # descriptors then walk ascending addresses (better DRAM locality)
    order = np.lexsort((idx16, grp))
    starts = np.zeros(NC * NPC * 2 + 1, dtype=np.int64)
    starts[1:] = np.cumsum(cnt2)
    rank = np.arange(E, dtype=np.int64) - starts[grp[order]]

    c_e = core[order]
    p_e = ploc[order]
    h_e = half[order]
    tl_e = tl[order]
    i_e = idx16[order]

    k_e = rank // 128                    # bin index within (pair, half)
    s_e = rank % 128                     # slot within bin (s')
    b_e = np.where(h_e == 0, k_e, K_LO + k_e)  # bin within pair, 0..NB-1

    # G column index within the group-flat layout: lo bins of all pairs of a
    # group first, then hi bins (each region contiguous for one gather call).
    # gcol(pair_in_group pl, h, k) = pl*K_LO + k            (h == 0)
    #                              = GP*K_LO + pl*K_HI + k  (h == 1)

    # gather idx streams, full capacity (pads gather row 0: finite dummy
    # data, neutralized by zero one-hot columns). Stream order must match
    # the G layout: per group, (pair, k, slot) for lo then same for hi.
    lo_s = np.zeros((NC, NPC * K_LO * 128), dtype=np.int16)
    hi_s = np.zeros((NC, NPC * K_HI * 128), dtype=np.int16)
    m0 = h_e == 0
    lo_s[c_e[m0], (p_e[m0] * K_LO + k_e[m0]) * 128 + s_e[m0]] = i_e[m0]
    m1 = ~m0
    hi_s[c_e[m1], (p_e[m1] * K_HI + k_e[m1]) * 128 + s_e[m1]] = i_e[m1]

    def wrap16(stream2d):
        # [NC, L] -> [NC, 128, L//16]: wrapped in 16 partitions, replicated x8
        ncc, L = stream2d.shape
        w = stream2d.reshape(ncc, L // 16, 16).transpose(0, 2, 1)
        return np.ascontiguousarray(np.tile(w, (1, 8, 1)))

    lo_idx = wrap16(lo_s)
    hi_idx = wrap16(hi_s)

    # local target ids per (slot, pair*NB+bin); pads 255 (matches no iota).
    # fp32: tensor_scalar is_equal requires a float32 scalar operand.
    tl_bf = np.full((NC, 128, NPC * NB), 255.0, dtype=np.float32)
    tl_bf[c_e, s_e, p_e * NB + b_e] = tl_e.astype(np.float32)

    # transposed one-hot as packed bits: bit j of word w of (tl, pair*NB+bin)
    # is set iff the edge at slot s' = 8*j + w of that bin targets tl.
    bitsT = np.zeros((NC, 128, NPC * NB, 8), dtype=np.int64)
    j_e = s_e // 8
    w_e = s_e % 8
    np.bitwise_or.at(
        bitsT, (c_e, tl_e, p_e * NB + b_e, w_e), np.int64(1) << j_e)
    bitsT = bitsT.astype(np.uint16).view(np.int16).reshape(NC, 128, -1)

    iota_mat = np.tile(np.arange(128, dtype=np.float32), (128, 1)).astype(BF16_NP)
    ident = np.eye(128, dtype=np.float32).astype(BF16_NP)

    whj = np.concatenate([weight, weight @ a2], axis=1).astype(BF16_NP)
    wi = (weight @ a1).astype(BF16_NP)

    base = np.zeros((IN_CH, N_PAD), dtype=np.float32)
    base[:, :N] = nodes_ft.T

    bias_sb = np.tile(bias[None, :], (128, 1)).astype(np.float32)

    in_maps = []
    for c in range(NC):
        nftT = np.ascontiguousarray(np.roll(base, -c * NSHARD, axis=1))
        in_maps.append({
            "nodes_ftT": nftT.astype(BF16_NP),
            "whj": whj, "wi": wi,
            "iota_mat": iota_mat, "ident": ident,
            "lo_idx": lo_idx[c], "hi_idx": hi_idx[c],
            "tl_bf": tl_bf[c], "bitsT": bitsT[c],
            "bias_sb": bias_sb,
        })
    meta = dict(K_LO=K_LO, K_HI=K_HI, NB=NB)
    return in_maps, meta


def _build_program(cfg, K_LO, K_HI, phase_limit="full", repeat=1):
    import concourse.bacc as bacc
    import concourse.bass as bass  # noqa: F401
    import concourse.mybir as mybir
    import concourse.tile as tile

    BF16 = mybir.dt.bfloat16
    F32 = mybir.dt.float32
    I16 = mybir.dt.int16
    ALU = mybir.AluOpType
    ACT = mybir.ActivationFunctionType

    NPC, NSHARD, HL = cfg["NPC"], cfg["NSHARD"], cfg["HL"]
    N_PAD = cfg["N_PAD"]
    T_TILES = cfg["T_TILES"]
    NG = cfg["NG"]
    NB = K_LO + K_HI
    GCOLS = GP * NB  # G columns per group

    LEAKY_MODE = os.environ.get("GAT_LEAKY", "prelu")

    do_build = phase_limit != "noop"
    do_gather = phase_limit in ("gather", "nomm", "full")
    do_dve = phase_limit in ("nomm", "full")
    do_mm = phase_limit == "full"

    # per-half AIPE/S psum chunking: chunks of <=5 bins (one 2KB psum bank
    # holds [128, 5, HC] f32 = 1280B per partition)
    def half_chunks(k):
        out, b0 = [], 0
        while b0 < k:
            step = min(5, k - b0)
            out.append((b0, b0 + step))
            b0 += step
        return out

    CH = [(0, k0, k1) for (k0, k1) in half_chunks(K_LO)] + \
         [(1, k0, k1) for (k0, k1) in half_chunks(K_HI)]

    def gcol(pl, h, k):
        return pl * K_LO + k if h == 0 else GP * K_LO + pl * K_HI + k

    nc = bacc.Bacc("TRN2", target_bir_lowering=False, debug=False,
                   num_swdge_queues=4)

    nodes_ftT = nc.dram_tensor("nodes_ftT", [IN_CH, N_PAD], BF16, kind="ExternalInput")
    whj_d = nc.dram_tensor("whj", [IN_CH, 2 * HC], BF16, kind="ExternalInput")
    wi_d = nc.dram_tensor("wi", [IN_CH, HC], BF16, kind="ExternalInput")
    iota_d = nc.dram_tensor("iota_mat", [128, 128], BF16, kind="ExternalInput")
    ident_d = nc.dram_tensor("ident", [128, 128], BF16, kind="ExternalInput")
    loidx_d = nc.dram_tensor("lo_idx", [128, NPC * K_LO * 8], I16, kind="ExternalInput")
    hiidx_d = nc.dram_tensor("hi_idx", [128, NPC * K_HI * 8], I16, kind="ExternalInput")
    tl_d = nc.dram_tensor("tl_bf", [128, NPC * NB], F32, kind="ExternalInput")
    bits_d = nc.dram_tensor("bitsT", [128, NPC * NB * 8], I16, kind="ExternalInput")
    bias_d = nc.dram_tensor("bias_sb", [128, HC], F32, kind="ExternalInput")
    out_d = nc.dram_tensor("out", [NSHARD, HC], F32, kind="ExternalOutput")

    hj_lo = nc.dram_tensor("hj_lo", [HL, 2 * HC], BF16, kind="Internal")
    hj_hi = nc.dram_tensor("hj_hi", [HL, 2 * HC], BF16, kind="Internal")

    with tile.TileContext(nc) as tc, ExitStack() as ctx:
        const_pool = ctx.enter_context(tc.tile_pool(name="const", bufs=1))
        stream_pool = ctx.enter_context(tc.tile_pool(name="streams", bufs=1))
        a_in = ctx.enter_context(tc.tile_pool(name="a_in", bufs=3))
        a_ps = ctx.enter_context(tc.tile_pool(name="a_ps", bufs=2, space="PSUM"))
        a_st = ctx.enter_context(tc.tile_pool(name="a_st", bufs=3))
        ai_pool = ctx.enter_context(tc.tile_pool(name="aip", bufs=1))
        g_pool = ctx.enter_context(tc.tile_pool(
            name="gp", bufs=int(os.environ.get("GAT_GBUFS", "4"))))
        oh_pool = ctx.enter_context(tc.tile_pool(name="ohp", bufs=4))
        oht_pool = ctx.enter_context(tc.tile_pool(name="ohtp", bufs=3))
        tmp_pool = ctx.enter_context(tc.tile_pool(name="tmpp", bufs=2))
        s_ps = ctx.enter_context(tc.tile_pool(name="sps", bufs=3, space="PSUM"))
        iota_ps_pool = ctx.enter_context(
            tc.tile_pool(name="iops", bufs=1, space="PSUM"))
        l_pool = ctx.enter_context(tc.tile_pool(name="lp", bufs=3))
        mm_pool = ctx.enter_context(tc.tile_pool(name="mmps", bufs=2, space="PSUM"))
        fl_pool = ctx.enter_context(tc.tile_pool(name="fl", bufs=1))

        whj_sb = const_pool.tile([IN_CH, 2 * HC], BF16)
        nc.sync.dma_start(whj_sb[:], whj_d[:])
        wi_sb = const_pool.tile([IN_CH, HC], BF16)
        nc.sync.dma_start(wi_sb[:], wi_d[:])
        iota_sb = const_pool.tile([128, 128], BF16)
        nc.sync.dma_start(iota_sb[:], iota_d[:])
        ident_sb = const_pool.tile([128, 128], BF16)
        nc.sync.dma_start(ident_sb[:], ident_d[:])
        bias_sb = const_pool.tile([128, HC], F32)
        nc.sync.dma_start(bias_sb[:], bias_d[:])

        # idx/aux streams: loaded once up-front
        lo_sb = stream_pool.tile([128, NPC * K_LO * 8], I16)
        nc.sync.dma_start(lo_sb[:], loidx_d[:])
        hi_sb = stream_pool.tile([128, NPC * K_HI * 8], I16)
        nc.sync.dma_start(hi_sb[:], hiidx_d[:])
        tl_sb = stream_pool.tile([128, NPC * NB], F32)
        nc.sync.dma_start(tl_sb[:], tl_d[:])
        bt_sb = stream_pool.tile([128, NPC * NB, 8], I16)
        nc.sync.dma_start(bt_sb[:].rearrange("p a b -> p (a b)"), bits_d[:])

        ai_sb = ai_pool.tile([128, NPC, HC], BF16)

        # iota in PSUM (fp32): the per-bin OH is_equal then reads no SBUF
        # port (PSUM has its own path), so it cannot lock GPSIMD out of the
        # shared SBUF port pair while SWDGE generates gather descriptors.
        OH_PSUM = bool(int(os.environ.get("GAT_OHPS", "1")))
        iota_ps = iota_ps_pool.tile([128, 128], F32)
        nc.vector.tensor_copy(iota_ps[:], iota_sb[:])

        def emit_once(rep):
            # ---- Phase A: hj tables in 8-tile chunks
            NCH = T_TILES // 8
            for ch in range(NCH if do_build else 0):
                t0 = 8 * ch
                nf = a_in.tile([128, 8, 128], BF16, tag="nf", name="nf")
                nc.sync.dma_start(
                    nf[:].rearrange("p a b -> p (a b)"),
                    nodes_ftT[:, 128 * t0:128 * (t0 + 8)])
                st = a_st.tile([128, 8, 2 * HC], BF16, tag="ast", name="ast")
                for half4 in range(2):
                    ps = a_ps.tile([128, 4, 2 * HC], F32, tag="aps", name="aps")
                    for j in range(4):
                        nc.tensor.matmul(ps[:, j, :], nf[:, 4 * half4 + j, :],
                                         whj_sb[:], start=True, stop=True)
                    if (2 * ch + half4) % 2 == 0:
                        nc.vector.tensor_copy(
                            st[:, 4 * half4:4 * half4 + 4, :], ps[:])
                    else:
                        nc.scalar.copy(
                            st[:, 4 * half4:4 * half4 + 4, :], ps[:])
                rbase = 1024 * ch
                if rbase + 1024 <= HL:
                    nc.sync.dma_start(
                        hj_lo[rbase:rbase + 1024, :]
                        .rearrange("(a p) b -> p a b", p=128), st[:])
                elif rbase >= HL:
                    nc.sync.dma_start(
                        hj_hi[rbase - HL:rbase - HL + 1024, :]
                        .rearrange("(a p) b -> p a b", p=128), st[:])
                else:
                    n_lo = (HL - rbase) // 128
                    nc.sync.dma_start(
                        hj_lo[rbase:HL, :]
                        .rearrange("(a p) b -> p a b", p=128),
                        st[:, 0:n_lo, :])
                    nc.sync.dma_start(
                        hj_hi[0:rbase + 1024 - HL, :]
                        .rearrange("(a p) b -> p a b", p=128),
                        st[:, n_lo:8, :])

            # ---- Phase A2: ai for own shard (re-read first NPC tiles)
            for ch in range(7 if do_build else 0):
                nf2 = a_in.tile([128, 7, 128], BF16, tag="nf2", name="nf2")
                nc.sync.dma_start(
                    nf2[:].rearrange("p a b -> p (a b)"),
                    nodes_ftT[:, 128 * 7 * ch:128 * 7 * (ch + 1)])
                ps2 = a_ps.tile([128, 7, HC], F32, tag="aps", name="a2ps")
                for j in range(7):
                    nc.tensor.matmul(ps2[:, j, :], nf2[:, j, :], wi_sb[:],
                                     start=True, stop=True)
                if ch % 2 == 0:
                    nc.scalar.copy(ai_sb[:, 7 * ch:7 * (ch + 1), :], ps2[:])
                else:
                    nc.vector.tensor_copy(ai_sb[:, 7 * ch:7 * (ch + 1), :], ps2[:])

            # ---- Phase B
            stage_n = fl_pool.tile([128, NPC * HC], F32, tag="sn", name="sn")
            stage_d = fl_pool.tile([128, NPC * HC], F32, tag="sd", name="sd")

            tiles = {}
            gtiles = {}

            def gather_group(g):
                p0 = g * GP
                G = g_pool.tile([128, GCOLS, 128], BF16, tag="G", name="G")
                SP = bool(int(os.environ.get("GAT_SP", "0")))
                SPLIT = int(os.environ.get("GAT_SPLIT", "1"))
                calls = []
                np_ = min(GP, NPC - p0)
                for h, K, tab, sb, base in (
                        (0, K_LO, hj_lo, lo_sb, 0),
                        (1, K_HI, hj_hi, hi_sb, GP * K_LO)):
                    nbins = np_ * K  # 128-idx bins in this half
                    per = max(1, nbins // SPLIT)
                    b0 = 0
                    for s in range(SPLIT):
                        b1 = nbins if s == SPLIT - 1 else min(nbins, b0 + per)
                        if b1 > b0:
                            calls.append((tab, sb, p0 * K * 8, base, b0, b1, K))
                        b0 = b1
                for ci, (tab, sb, off8, base, b0, b1, K) in enumerate(calls):
                    nc.gpsimd.dma_gather(
                        out_ap=G[:, base + b0:base + b1, :], in_ap=tab[:],
                        idxs_ap=sb[:, off8 + b0 * 8:off8 + b1 * 8],
                        num_idxs=(b1 - b0) * 128,
                        num_idxs_reg=(b1 - b0) * 128, elem_size=2 * HC,
                        queue_num=(len(calls) * g + ci) % 4,
                        single_packet=SP)
                gtiles[g] = G

            def produce_oh(p):
                # OH [slot, NB, tl] via per-bin tensor_scalar is_equal.
                # PSUM-sourced iota -> 1x mode but no shared-port lock.
                OH = oh_pool.tile([128, NB, 128], BF16, tag="OH", name="OH")
                src0 = iota_ps if OH_PSUM else iota_sb
                for b in range(NB):
                    nc.vector.tensor_scalar(
                        out=OH[:, b, :], in0=src0[:],
                        scalar1=tl_sb[:, p * NB + b:p * NB + b + 1],
                        scalar2=None, op0=ALU.is_equal)
                # OHT [tl, NB, slot]: 16 tensor_scalar AND unpacks (int16,
                # 4x) then one big != 0 pass (also does the int16->bf16
                # cast; bitwise TS ops cannot cast)
                TMP = tmp_pool.tile([128, NB, 16, 8], I16, tag="TMP", name="TMP")
                bt_p = bt_sb[:, p * NB:(p + 1) * NB, :]
                for j in range(16):
                    nc.vector.tensor_scalar(
                        out=TMP[:, :, j, :], in0=bt_p,
                        scalar1=int(1 << j) if j < 15 else -32768,
                        scalar2=None, op0=ALU.bitwise_and)
                OHT = oht_pool.tile([128, NB, 16, 8], BF16, tag="OHT", name="OHT")
                nc.vector.tensor_scalar(
                    out=OHT[:], in0=TMP[:], scalar1=0, scalar2=None,
                    op0=ALU.not_equal)
                tiles[p] = {"OH": OH, "OHT": OHT}

            def produce_s(p):
                # per chunk: PSUM accumulates OHT_b.T@ai + I.T@G_aj, then
                # ACT: L = lrelu(ps); x = exp(L) -> G aj half
                G = gtiles[p // GP]
                pl = p % GP
                OHT2 = tiles[p]["OHT"][:].rearrange("p a j w -> p (a j w)")
                # a single chunk-wide identity matmul with a strided moving
                # operand miscomputes (rel err 0.38) -- keep per-bin adds
                IDMERGE = bool(int(os.environ.get("GAT_IDMERGE", "0")))
                for (h, k0, k1) in CH:
                    ps_s = s_ps.tile([128, 5, HC], F32, tag="sps",
                                     name=f"ps{h}{k0}")
                    for k in range(k0, k1):
                        b = h * K_LO + k
                        nc.tensor.matmul(
                            ps_s[:, k - k0, :],
                            OHT2[:, b * 128:(b + 1) * 128],
                            ai_sb[:, p, :],
                            start=True, stop=IDMERGE and False)
                        if not IDMERGE:
                            nc.tensor.matmul(
                                ps_s[:, k - k0, :], ident_sb[:],
                                G[:, gcol(pl, h, k), HC:2 * HC],
                                start=False, stop=True)
                    if IDMERGE:
                        # one identity matmul accumulates the whole chunk's aj
                        c0 = gcol(pl, h, k0)
                        nc.tensor.matmul(
                            ps_s[:, 0:k1 - k0, :],
                            ident_sb[:],
                            G[:, c0:c0 + (k1 - k0), HC:2 * HC],
                            start=False, stop=True)
                    gdst = G[:, gcol(pl, h, k0):gcol(pl, h, k1 - 1) + 1,
                             HC:2 * HC]
                    if LEAKY_MODE == "prelu":
                        L = l_pool.tile([128, 5, HC], BF16, tag="L", name="L")
                        nc.scalar.activation(
                            L[:, 0:k1 - k0, :], ps_s[:, 0:k1 - k0, :],
                            ACT.Prelu, alpha=NEG_SLOPE)
                        nc.scalar.activation(
                            gdst, L[:, 0:k1 - k0, :], ACT.Exp)
                    else:
                        # x = exp(leaky(s)) = max(exp(0.2 s), exp(s))
                        L = l_pool.tile([128, 5, HC], BF16, tag="L", name="L")
                        nc.scalar.activation(
                            L[:, 0:k1 - k0, :], ps_s[:, 0:k1 - k0, :],
                            ACT.Exp, scale=NEG_SLOPE)
                        nc.scalar.activation(
                            gdst, ps_s[:, 0:k1 - k0, :], ACT.Exp)
                        nc.vector.tensor_tensor(
                            out=gdst, in0=gdst, in1=L[:, 0:k1 - k0, :],
                            op=ALU.max)

            def consume_mm(p):
                # y = h*x ; MM += OH_b.T @ [y|x] ; flush
                G = gtiles[p // GP]
                pl = p % GP
                t = tiles.pop(p)
                OH = t["OH"]
                nc.vector.tensor_tensor(
                    out=G[:, pl * K_LO:(pl + 1) * K_LO, 0:HC],
                    in0=G[:, pl * K_LO:(pl + 1) * K_LO, 0:HC],
                    in1=G[:, pl * K_LO:(pl + 1) * K_LO, HC:2 * HC],
                    op=ALU.mult)
                h0 = GP * K_LO + pl * K_HI
                nc.vector.tensor_tensor(
                    out=G[:, h0:h0 + K_HI, 0:HC],
                    in0=G[:, h0:h0 + K_HI, 0:HC],
                    in1=G[:, h0:h0 + K_HI, HC:2 * HC],
                    op=ALU.mult)
                MM = mm_pool.tile([128, 2 * HC], F32, tag="MM", name="MM")
                bi = 0
                for h, K in ((0, K_LO), (1, K_HI)):
                    for k in range(K):
                        nc.tensor.matmul(
                            MM[:], t["OH"][:, h * K_LO + k, :],
                            G[:, gcol(pl, h, k), :],
                            start=(bi == 0), stop=(bi == NB - 1))
                        bi += 1
                nc.scalar.copy(stage_n[:, HC * p:HC * (p + 1)], MM[:, 0:HC])
                nc.scalar.copy(stage_d[:, HC * p:HC * (p + 1)], MM[:, HC:2 * HC])

            # software pipeline: gathers prefetch LOOKAHEAD groups ahead of
            # the compute; per-pair stages offset one iteration apart
            LOOKAHEAD = max(0, int(os.environ.get("GAT_GBUFS", "4")) - 2)
            for it in range((NPC + 3) if do_gather else 0):
                if it == 0:
                    for ga in range(min(LOOKAHEAD + 1, NG)):
                        gather_group(ga)
                elif it % GP == 0 and it // GP + LOOKAHEAD < NG:
                    gather_group(it // GP + LOOKAHEAD)
                if do_dve and it < NPC:
                    produce_oh(it)
                if do_dve and 0 <= it - 1 < NPC:
                    produce_s(it - 1)
                if do_dve and do_mm and 0 <= it - 3 < NPC:
                    consume_mm(it - 3)

            # ---- Phase C: out = numer / (denom + eps) + bias
            if not do_mm:
                nc.vector.memset(stage_n[:], 0.0)
                nc.vector.memset(stage_d[:], 1.0)
            nc.vector.tensor_scalar(
                out=stage_d[:], in0=stage_d[:], scalar1=1e-16, scalar2=None,
                op0=ALU.add)
            lnd = fl_pool.tile([128, NPC * HC], F32, tag="lnd", name="lnd")
            nc.scalar.activation(lnd[:], stage_d[:], ACT.Ln)
            nc.scalar.activation(lnd[:], lnd[:], ACT.Exp, scale=-1.0)
            nc.vector.tensor_tensor(out=stage_n[:], in0=stage_n[:],
                                    in1=lnd[:], op=ALU.mult)
            nc.vector.tensor_tensor(
                out=stage_n[:].rearrange("p (a c) -> p a c", c=HC),
                in0=stage_n[:].rearrange("p (a c) -> p a c", c=HC),
                in1=bias_sb[:].rearrange("p (a c) -> p a c", a=1)
                    .broadcast_to([128, NPC, HC]),
                op=ALU.add)

            out_view = out_d[:].rearrange("(pr p) c -> p pr c", p=128)
            st_view = stage_n[:].rearrange("p (pr c) -> p pr c", c=HC)
            nc.sync.dma_start(out_view, st_view)

        for rep in range(repeat):
            emit_once(rep)
            if repeat > 1:
                tc.strict_bb_all_engine_barrier()

    nc.compile()
    return nc


def kernel(**inputs):
    cfg = _cfg()
    in_maps, meta = _prep(inputs, cfg)
    nc = _build_program(cfg, meta["K_LO"], meta["K_HI"])

    from concourse import bass_utils
    res = bass_utils.run_bass_kernel_spmd(
        nc, in_maps, core_ids=list(range(cfg["NC"])),
        trace=bool(int(os.environ.get("GAT_TRACE", "0"))),
    )
    kernel.last_result = res
    kernel.last_ctx = (nc, in_maps, cfg)

    NSHARD = cfg["NSHARD"]
    out_full = np.zeros((cfg["NC"] * NSHARD, HC), dtype=np.float32)
    for c in range(cfg["NC"]):
        out_full[c * NSHARD:(c + 1) * NSHARD] = res.results[c]["out"]
    return out_full[:cfg["N"]]


# revision 45
# speedup vs baseline: 2.5157x; 2.5157x over previous
"""Bass/Trainium2 kernel for nn_BitGatConv (GAT-style message passing), V3.

Self-contained: takes full inputs, shards edges by destination window-pair
across 8 NeuronCores (SPMD, one program), returns the full [N, HC] output.

V3 vs V2 — driven by phase ablation (gather phase was +473 us of the 780 us
total, dominated by per-call SWDGE/HWDGE fixed overheads ~1-2.4 us x 392
calls) and the DVE perf-mode rules (broadcast operands force 1x mode):

  - Gathers are batched per GROUP of GP=7 destination pairs: 2 dma_gather
    calls per group (14 total vs 196), full-capacity idx streams (dummy
    row-0 pads, no -1 tails) so no G memset is needed.
  - All idx/aux streams are loaded in 4 up-front DMAs; phase A runs in
    8-tile chunks (1 read + 8 matmuls + 1 copy + 1 write per chunk).
  - One-hot builds avoid broadcast operands: OH (scatter one-hot,
    [slot,bin,tl]) is built with per-bin tensor_scalar is_equal against a
    materialized iota row (4x DVE mode, per-partition scalar = tl).
    OHT (transposed one-hot, [tl,bin,slot]) is unpacked from host-packed
    bitmasks with 16 dual-op tensor_scalar ops ((bits AND (1<<j)) != 0),
    also 4x; the bit layout matches slot s' via j = s'//8, w = s'%8.
  - s = att_i[tgt] + att_j[src] is formed ON PE: per bin, PSUM accumulates
    OHT_b.T @ ai_pair (att_i expand) + I.T @ G_aj (the gathered att_j),
    so DVE never touches fp32 PSUM. leaky relu + exp run on ACT (Lrelu
    reads PSUM natively, alpha=0.2; Exp writes back into G's aj half).
  - Per-pair DVE work is only: 18 OH builds + 16 OHT unpacks + y=h*x
    tensor_tensor (all 2-byte, stride-1, 2x/4x modes).

Algorithm per core (rotated node ids so all cores run the same program):
  Phase A: hj table rows [h | att_j] -> DRAM (lo/hi for int16 gather
    reach); ai_sb = nodes_ft @ (W@A1) for own shard -> SBUF.
  Phase B: per group: gather [h|aj] rows by src (full capacity); per pair:
    build OH/OHT; PE: ps_s = OHT_b.T@ai + aj; ACT: x = exp(lrelu(ps_s))
    -> G aj half; DVE: y = h*x -> G h half; PE: MM += OH_b.T @ [y|x].
  Phase C: out = numer / (denom + 1e-16) + bias.
  No segment-max subtraction: logits are bounded so exp is safe, and
  softmax is shift-free identical.
"""

import math
import os
import sys
from contextlib import ExitStack

import numpy as np

for _p in ("/opt/trn_rl_repo",):
    if _p not in sys.path:
        sys.path.insert(0, _p)

import ml_dtypes  # noqa: E402

BF16_NP = ml_dtypes.bfloat16

# ---------------------------------------------------------------------------
# Problem constants (hardcoded per contest rules)
N_NODES = 50000
N_EDGES = 800000
IN_CH = 128
HC = 64
NEG_SLOPE = 0.2
N_CORES = 8
PW = 128  # nodes per scatter pair (one-hot width)
GP = 4    # destination pairs per gather group


def _cfg(n_nodes=N_NODES, n_cores=N_CORES):
    npc = math.ceil(math.ceil(n_nodes / PW) / n_cores)  # pairs per core
    nshard = npc * PW
    n_pad = n_cores * nshard
    return dict(
        N=n_nodes, NC=n_cores, NPC=npc, NSHARD=nshard, N_PAD=n_pad,
        HL=n_pad // 2, T_TILES=n_pad // 128, NG=math.ceil(npc / GP),
    )


def _prep(inputs, cfg):
    """Host-side preprocessing: shard + pad + index building (numpy only)."""
    NC, NPC, NSHARD, N_PAD, HL = (
        cfg["NC"], cfg["NPC"], cfg["NSHARD"], cfg["N_PAD"], cfg["HL"])
    N = cfg["N"]

    nodes_ft = np.asarray(inputs["nodes_ft"], dtype=np.float32)
    adj = np.asarray(inputs["adj_list"])
    weight = np.asarray(inputs["weight"], dtype=np.float32)
    a1 = np.asarray(inputs["att_layer_1"], dtype=np.float32)
    a2 = np.asarray(inputs["att_layer_2"], dtype=np.float32)
    bias = np.asarray(inputs["bias"], dtype=np.float32)

    tgt = adj[0].astype(np.int64)
    src = adj[1].astype(np.int64)
    E = tgt.shape[0]

    pair = tgt // PW
    core = pair // NPC
    ploc = pair % NPC
    tl = (tgt % PW).astype(np.int64)

    src_rot = (src - core * NSHARD) % N_PAD
    half = (src_rot >= HL).astype(np.int64)
    idx16 = (src_rot - half * HL).astype(np.int16)

    grp = (core * NPC + ploc) * 2 + half
    cnt2 = np.bincount(grp, minlength=NC * NPC * 2)
    K_LO = max(1, int(math.ceil(cnt2[0::2].max() / 128.0)))
    K_HI = max(1, int(math.ceil(cnt2[1::2].max() / 128.0)))
    NB = K_LO + K_HI

    # sort each (core, pair, half) segment by source row: the gather's DMA
    # descriptors then walk ascending addresses (better DRAM locality)
    order = np.lexsort((idx16, grp))
    starts = np.zeros(NC * NPC * 2 + 1, dtype=np.int64)
    starts[1:] = np.cumsum(cnt2)
    rank = np.arange(E, dtype=np.int64) - starts[grp[order]]

    c_e = core[order]
    p_e = ploc[order]
    h_e = half[order]
    tl_e = tl[order]
    i_e = idx16[order]

    k_e = rank // 128                    # bin index within (pair, half)
    s_e = rank % 128                     # slot within bin (s')
    b_e = np.where(h_e == 0, k_e, K_LO + k_e)  # bin within pair, 0..NB-1

    # G column index within the group-flat layout: lo bins of all pairs of a
    # group first, then hi bins (each region contiguous for one gather call).
    # gcol(pair_in_group pl, h, k) = pl*K_LO + k            (h == 0)
    #                              = GP*K_LO + pl*K_HI + k  (h == 1)

    # gather idx streams, full capacity (pads gather row 0: finite dummy
    # data, neutralized by zero one-hot columns). Stream order must match
    # the G layout: per group, (pair, k, slot) for lo then same for hi.
    lo_s = np.zeros((NC, NPC * K_LO * 128), dtype=np.int16)
    hi_s = np.zeros((NC, NPC * K_HI * 128), dtype=np.int16)
    m0 = h_e == 0
    lo_s[c_e[m0], (p_e[m0] * K_LO + k_e[m0]) * 128 + s_e[m0]] = i_e[m0]
    m1 = ~m0
    hi_s[c_e[m1], (p_e[m1] * K_HI + k_e[m1]) * 128 + s_e[m1]] = i_e[m1]

    def wrap16(stream2d):
        # [NC, L] -> [NC, 128, L//16]: wrapped in 16 partitions, replicated x8
        ncc, L = stream2d.shape
        w = stream2d.reshape(ncc, L // 16, 16).transpose(0, 2, 1)
        return np.ascontiguousarray(np.tile(w, (1, 8, 1)))

    lo_idx = wrap16(lo_s)
    hi_idx = wrap16(hi_s)

    # one-hots in fp8 (exact 0/1), host-built, streamed to SBUF by HWDGE:
    # oh8[slot, pair*NB+bin, tl], oht8[tl, pair*NB+bin, slot]. fp8
    # stationary x bf16 moving matmuls are supported and exact, and keeping
    # the one-hot construction off DVE lets SWDGE gather generation run
    # unblocked (DVE activity does not overlap SWDGE on this part).
    FP8_NP = ml_dtypes.float8_e4m3
    oh8 = np.zeros((NC, 128, NPC * NB, 128), dtype=FP8_NP)
    oh8[c_e, s_e, p_e * NB + b_e, tl_e] = 1.0
    oht8 = np.zeros((NC, 128, NPC * NB, 128), dtype=FP8_NP)
    oht8[c_e, tl_e, p_e * NB + b_e, s_e] = 1.0
    oh8 = oh8.reshape(NC, 128, -1)
    oht8 = oht8.reshape(NC, 128, -1)

    ident = np.eye(128, dtype=np.float32).astype(BF16_NP)

    whj = np.concatenate([weight, weight @ a2], axis=1).astype(BF16_NP)
    wi = (weight @ a1).astype(BF16_NP)

    base = np.zeros((IN_CH, N_PAD), dtype=np.float32)
    base[:, :N] = nodes_ft.T

    bias_sb = np.tile(bias[None, :], (128, 1)).astype(np.float32)

    in_maps = []
    for c in range(NC):
        nftT = np.ascontiguousarray(np.roll(base, -c * NSHARD, axis=1))
        in_maps.append({
            "nodes_ftT": nftT.astype(BF16_NP),
            "whj": whj, "wi": wi, "ident": ident,
            "lo_idx": lo_idx[c], "hi_idx": hi_idx[c],
            "oh8": oh8[c], "oht8": oht8[c],
            "bias_sb": bias_sb,
        })
    meta = dict(K_LO=K_LO, K_HI=K_HI, NB=NB)
    return in_maps, meta


def _build_program(cfg, K_LO, K_HI, phase_limit="full", repeat=1):
    import concourse.bacc as bacc
    import concourse.bass as bass  # noqa: F401
    import concourse.mybir as mybir
    import concourse.tile as tile

    BF16 = mybir.dt.bfloat16
    FP8 = mybir.dt.float8e4
    F32 = mybir.dt.float32
    I16 = mybir.dt.int16
    ALU = mybir.AluOpType
    ACT = mybir.ActivationFunctionType

    NPC, NSHARD, HL = cfg["NPC"], cfg["NSHARD"], cfg["HL"]
    N_PAD = cfg["N_PAD"]
    T_TILES = cfg["T_TILES"]
    NG = cfg["NG"]
    NB = K_LO + K_HI
    GCOLS = GP * NB  # G columns per group

    LEAKY_MODE = os.environ.get("GAT_LEAKY", "prelu")

    _levels = ["noop", "build", "gather", "dve1", "pe1", "nomm", "full"]
    _lv = _levels.index(phase_limit)
    do_build = _lv >= 1
    do_gather = _lv >= 2
    do_oh = _lv >= 3      # one-hot builds (DVE)
    do_ps = _lv >= 4      # produce_s matmuls (PE)
    do_exp = _lv >= 5     # prelu/exp (ACT)
    do_dve = _lv >= 3
    do_mm = _lv >= 6

    # per-half AIPE/S psum chunking: chunks of <=5 bins (one 2KB psum bank
    # holds [128, 5, HC] f32 = 1280B per partition)
    def half_chunks(k):
        out, b0 = [], 0
        while b0 < k:
            step = min(5, k - b0)
            out.append((b0, b0 + step))
            b0 += step
        return out

    CH = [(0, k0, k1) for (k0, k1) in half_chunks(K_LO)] + \
         [(1, k0, k1) for (k0, k1) in half_chunks(K_HI)]

    def gcol(pl, h, k):
        return pl * K_LO + k if h == 0 else GP * K_LO + pl * K_HI + k

    nc = bacc.Bacc("TRN2", target_bir_lowering=False, debug=False,
                   num_swdge_queues=4)

    nodes_ftT = nc.dram_tensor("nodes_ftT", [IN_CH, N_PAD], BF16, kind="ExternalInput")
    whj_d = nc.dram_tensor("whj", [IN_CH, 2 * HC], BF16, kind="ExternalInput")
    wi_d = nc.dram_tensor("wi", [IN_CH, HC], BF16, kind="ExternalInput")
    ident_d = nc.dram_tensor("ident", [128, 128], BF16, kind="ExternalInput")
    loidx_d = nc.dram_tensor("lo_idx", [128, NPC * K_LO * 8], I16, kind="ExternalInput")
    hiidx_d = nc.dram_tensor("hi_idx", [128, NPC * K_HI * 8], I16, kind="ExternalInput")
    oh8_d = nc.dram_tensor("oh8", [128, NPC * NB * 128], FP8, kind="ExternalInput")
    oht8_d = nc.dram_tensor("oht8", [128, NPC * NB * 128], FP8, kind="ExternalInput")
    bias_d = nc.dram_tensor("bias_sb", [128, HC], F32, kind="ExternalInput")
    out_d = nc.dram_tensor("out", [NSHARD, HC], F32, kind="ExternalOutput")

    hj_lo = nc.dram_tensor("hj_lo", [HL, 2 * HC], BF16, kind="Internal")
    hj_hi = nc.dram_tensor("hj_hi", [HL, 2 * HC], BF16, kind="Internal")

    with tile.TileContext(nc) as tc, ExitStack() as ctx:
        const_pool = ctx.enter_context(tc.tile_pool(name="const", bufs=1))
        stream_pool = ctx.enter_context(tc.tile_pool(name="streams", bufs=1))
        a_in = ctx.enter_context(tc.tile_pool(name="a_in", bufs=3))
        a_ps = ctx.enter_context(tc.tile_pool(name="a_ps", bufs=2, space="PSUM"))
        a_st = ctx.enter_context(tc.tile_pool(name="a_st", bufs=3))
        ai_pool = ctx.enter_context(tc.tile_pool(name="aip", bufs=1))
        GROUP_SCHED = os.environ.get("GAT_SCHED", "pair") == "group"
        g_pool = ctx.enter_context(tc.tile_pool(
            name="gp", bufs=int(os.environ.get(
                "GAT_GBUFS", "3" if GROUP_SCHED else "4"))))
        oh_pool = ctx.enter_context(tc.tile_pool(name="ohp", bufs=3))
        oht_pool = ctx.enter_context(tc.tile_pool(name="ohtp", bufs=3))
        s_ps = ctx.enter_context(tc.tile_pool(name="sps", bufs=3, space="PSUM"))
        l_pool = ctx.enter_context(tc.tile_pool(name="lp", bufs=3))
        mm_pool = ctx.enter_context(tc.tile_pool(name="mmps", bufs=2, space="PSUM"))
        fl_pool = ctx.enter_context(tc.tile_pool(name="fl", bufs=1))

        whj_sb = const_pool.tile([IN_CH, 2 * HC], BF16)
        nc.sync.dma_start(whj_sb[:], whj_d[:])
        wi_sb = const_pool.tile([IN_CH, HC], BF16)
        nc.sync.dma_start(wi_sb[:], wi_d[:])
        ident_sb = const_pool.tile([128, 128], BF16)
        nc.sync.dma_start(ident_sb[:], ident_d[:])
        bias_sb = const_pool.tile([128, HC], F32)
        nc.sync.dma_start(bias_sb[:], bias_d[:])

        # idx/aux streams: loaded once up-front
        lo_sb = stream_pool.tile([128, NPC * K_LO * 8], I16)
        nc.sync.dma_start(lo_sb[:], loidx_d[:])
        hi_sb = stream_pool.tile([128, NPC * K_HI * 8], I16)
        nc.sync.dma_start(hi_sb[:], hiidx_d[:])
        ai_sb = ai_pool.tile([128, NPC, HC], BF16)

        def emit_once(rep):
            # ---- Phase A: hj tables in 8-tile chunks
            NCH = T_TILES // 8
            for ch in range(NCH if do_build else 0):
                t0 = 8 * ch
                nf = a_in.tile([128, 8, 128], BF16, tag="nf", name="nf")
                nc.sync.dma_start(
                    nf[:].rearrange("p a b -> p (a b)"),
                    nodes_ftT[:, 128 * t0:128 * (t0 + 8)])
                st = a_st.tile([128, 8, 2 * HC], BF16, tag="ast", name="ast")
                for half4 in range(2):
                    ps = a_ps.tile([128, 4, 2 * HC], F32, tag="aps", name="aps")
                    for j in range(4):
                        nc.tensor.matmul(ps[:, j, :], nf[:, 4 * half4 + j, :],
                                         whj_sb[:], start=True, stop=True)
                    if (2 * ch + half4) % 2 == 0:
                        nc.vector.tensor_copy(
                            st[:, 4 * half4:4 * half4 + 4, :], ps[:])
                    else:
                        nc.scalar.copy(
                            st[:, 4 * half4:4 * half4 + 4, :], ps[:])
                rbase = 1024 * ch
                if rbase + 1024 <= HL:
                    nc.sync.dma_start(
                        hj_lo[rbase:rbase + 1024, :]
                        .rearrange("(a p) b -> p a b", p=128), st[:])
                elif rbase >= HL:
                    nc.sync.dma_start(
                        hj_hi[rbase - HL:rbase - HL + 1024, :]
                        .rearrange("(a p) b -> p a b", p=128), st[:])
                else:
                    n_lo = (HL - rbase) // 128
                    nc.sync.dma_start(
                        hj_lo[rbase:HL, :]
                        .rearrange("(a p) b -> p a b", p=128),
                        st[:, 0:n_lo, :])
                    nc.sync.dma_start(
                        hj_hi[0:rbase + 1024 - HL, :]
                        .rearrange("(a p) b -> p a b", p=128),
                        st[:, n_lo:8, :])

            # ---- Phase A2: ai for own shard (re-read first NPC tiles)
            for ch in range(7 if do_build else 0):
                nf2 = a_in.tile([128, 7, 128], BF16, tag="nf2", name="nf2")
                nc.sync.dma_start(
                    nf2[:].rearrange("p a b -> p (a b)"),
                    nodes_ftT[:, 128 * 7 * ch:128 * 7 * (ch + 1)])
                ps2 = a_ps.tile([128, 7, HC], F32, tag="aps", name="a2ps")
                for j in range(7):
                    nc.tensor.matmul(ps2[:, j, :], nf2[:, j, :], wi_sb[:],
                                     start=True, stop=True)
                if ch % 2 == 0:
                    nc.scalar.copy(ai_sb[:, 7 * ch:7 * (ch + 1), :], ps2[:])
                else:
                    nc.vector.tensor_copy(ai_sb[:, 7 * ch:7 * (ch + 1), :], ps2[:])

            # ---- Phase B
            stage_n = fl_pool.tile([128, NPC * HC], F32, tag="sn", name="sn")
            stage_d = fl_pool.tile([128, NPC * HC], F32, tag="sd", name="sd")

            gtiles = {}
            ohtiles = {}

            def gather_group(g):
                p0 = g * GP
                G = g_pool.tile([128, GCOLS, 128], BF16, tag="G", name="G")
                SP = bool(int(os.environ.get("GAT_SP", "0")))
                SPLIT = int(os.environ.get("GAT_SPLIT", "1"))
                calls = []
                np_ = min(GP, NPC - p0)
                for h, K, tab, sb, base in (
                        (0, K_LO, hj_lo, lo_sb, 0),
                        (1, K_HI, hj_hi, hi_sb, GP * K_LO)):
                    nbins = np_ * K  # 128-idx bins in this half
                    per = max(1, nbins // SPLIT)
                    b0 = 0
                    for s in range(SPLIT):
                        b1 = nbins if s == SPLIT - 1 else min(nbins, b0 + per)
                        if b1 > b0:
                            calls.append((tab, sb, p0 * K * 8, base, b0, b1, K))
                        b0 = b1
                for ci, (tab, sb, off8, base, b0, b1, K) in enumerate(calls):
                    nc.gpsimd.dma_gather(
                        out_ap=G[:, base + b0:base + b1, :], in_ap=tab[:],
                        idxs_ap=sb[:, off8 + b0 * 8:off8 + b1 * 8],
                        num_idxs=(b1 - b0) * 128,
                        num_idxs_reg=(b1 - b0) * 128, elem_size=2 * HC,
                        queue_num=(len(calls) * g + ci) % 4,
                        single_packet=SP)
                gtiles[g] = G

            def load_onehots(g):
                # stream the group's fp8 one-hots in via HWDGE (no DVE work,
                # no SWDGE interference)
                p0 = g * GP
                np_ = min(GP, NPC - p0)
                OH = oh_pool.tile([128, GP * NB, 128], FP8, tag="OH", name="OH")
                nc.scalar.dma_start(
                    OH[:, 0:np_ * NB, :].rearrange("p a b -> p (a b)"),
                    oh8_d[:, p0 * NB * 128:(p0 + np_) * NB * 128])
                OHT = oht_pool.tile([128, GP * NB, 128], FP8, tag="OHT",
                                    name="OHT")
                nc.scalar.dma_start(
                    OHT[:, 0:np_ * NB, :].rearrange("p a b -> p (a b)"),
                    oht8_d[:, p0 * NB * 128:(p0 + np_) * NB * 128])
                ohtiles[g] = (OH, OHT)

            def produce_s(p):
                # per chunk: PSUM accumulates OHT_b.T@ai + I.T@G_aj, then
                # ACT: L = lrelu(ps); x = exp(L) -> G aj half
                G = gtiles[p // GP]
                pl = p % GP
                OHT2 = ohtiles[p // GP][1]
                # a single chunk-wide identity matmul with a strided moving
                # operand miscomputes (rel err 0.38) -- keep per-bin adds
                IDMERGE = bool(int(os.environ.get("GAT_IDMERGE", "0")))
                for (h, k0, k1) in CH:
                    ps_s = s_ps.tile([128, 5, HC], F32, tag="sps",
                                     name=f"ps{h}{k0}")
                    for k in range(k0, k1):
                        b = h * K_LO + k
                        nc.tensor.matmul(
                            ps_s[:, k - k0, :],
                            OHT2[:, pl * NB + b, :],
                            ai_sb[:, p, :],
                            start=True, stop=False)
                        if not IDMERGE:
                            nc.tensor.matmul(
                                ps_s[:, k - k0, :], ident_sb[:],
                                G[:, gcol(pl, h, k), HC:2 * HC],
                                start=False, stop=True)
                    if IDMERGE:
                        # one identity matmul accumulates the whole chunk's aj
                        c0 = gcol(pl, h, k0)
                        nc.tensor.matmul(
                            ps_s[:, 0:k1 - k0, :],
                            ident_sb[:],
                            G[:, c0:c0 + (k1 - k0), HC:2 * HC],
                            start=False, stop=True)
                    gdst = G[:, gcol(pl, h, k0):gcol(pl, h, k1 - 1) + 1,
                             HC:2 * HC]
                    if not do_exp:
                        continue
                    if LEAKY_MODE == "prelu":
                        L = l_pool.tile([128, 5, HC], BF16, tag="L", name="L")
                        nc.scalar.activation(
                            L[:, 0:k1 - k0, :], ps_s[:, 0:k1 - k0, :],
                            ACT.Prelu, alpha=NEG_SLOPE)
                        nc.scalar.activation(
                            gdst, L[:, 0:k1 - k0, :], ACT.Exp)
                    else:
                        # x = exp(leaky(s)) = max(exp(0.2 s), exp(s))
                        L = l_pool.tile([128, 5, HC], BF16, tag="L", name="L")
                        nc.scalar.activation(
                            L[:, 0:k1 - k0, :], ps_s[:, 0:k1 - k0, :],
                            ACT.Exp, scale=NEG_SLOPE)
                        nc.scalar.activation(
                            gdst, ps_s[:, 0:k1 - k0, :], ACT.Exp)
                        nc.vector.tensor_tensor(
                            out=gdst, in0=gdst, in1=L[:, 0:k1 - k0, :],
                            op=ALU.max)

            def consume_mm(p):
                # y = h*x ; MM += OH_b.T @ [y|x] ; flush
                G = gtiles[p // GP]
                pl = p % GP
                OH = ohtiles[p // GP][0]
                nc.vector.tensor_tensor(
                    out=G[:, pl * K_LO:(pl + 1) * K_LO, 0:HC],
                    in0=G[:, pl * K_LO:(pl + 1) * K_LO, 0:HC],
                    in1=G[:, pl * K_LO:(pl + 1) * K_LO, HC:2 * HC],
                    op=ALU.mult)
                h0 = GP * K_LO + pl * K_HI
                nc.vector.tensor_tensor(
                    out=G[:, h0:h0 + K_HI, 0:HC],
                    in0=G[:, h0:h0 + K_HI, 0:HC],
                    in1=G[:, h0:h0 + K_HI, HC:2 * HC],
                    op=ALU.mult)
                MM = mm_pool.tile([128, 2 * HC], F32, tag="MM", name="MM")
                bi = 0
                for h, K in ((0, K_LO), (1, K_HI)):
                    for k in range(K):
                        nc.tensor.matmul(
                            MM[:], OH[:, pl * NB + h * K_LO + k, :],
                            G[:, gcol(pl, h, k), :],
                            start=(bi == 0), stop=(bi == NB - 1))
                        bi += 1
                nc.scalar.copy(stage_n[:, HC * p:HC * (p + 1)], MM[:, 0:HC])
                nc.scalar.copy(stage_d[:, HC * p:HC * (p + 1)], MM[:, HC:2 * HC])

            # software pipeline: gathers prefetch LOOKAHEAD groups ahead of
            # the compute
            LOOKAHEAD = max(0, int(os.environ.get(
                "GAT_GBUFS", "3" if GROUP_SCHED else "4")) - 2)
            for it in range((NPC + 3) if do_gather else 0):
                if it == 0:
                    for ga in range(min(LOOKAHEAD + 1, NG)):
                        gather_group(ga)
                    if do_oh:
                        load_onehots(0)
                        if NG > 1:
                            load_onehots(1)
                elif it % GP == 0:
                    if it // GP + LOOKAHEAD < NG:
                        gather_group(it // GP + LOOKAHEAD)
                    if do_oh and it // GP + 1 < NG:
                        load_onehots(it // GP + 1)
                if do_ps and 0 <= it - 1 < NPC:
                    produce_s(it - 1)
                if do_mm and 0 <= it - 3 < NPC:
                    consume_mm(it - 3)

            # ---- Phase C: out = numer / (denom + eps) + bias
            if not do_mm:
                nc.vector.memset(stage_n[:], 0.0)
                nc.vector.memset(stage_d[:], 1.0)
            nc.vector.tensor_scalar(
                out=stage_d[:], in0=stage_d[:], scalar1=1e-16, scalar2=None,
                op0=ALU.add)
            lnd = fl_pool.tile([128, NPC * HC], F32, tag="lnd", name="lnd")
            nc.scalar.activation(lnd[:], stage_d[:], ACT.Ln)
            nc.scalar.activation(lnd[:], lnd[:], ACT.Exp, scale=-1.0)
            nc.vector.tensor_tensor(out=stage_n[:], in0=stage_n[:],
                                    in1=lnd[:], op=ALU.mult)
            nc.vector.tensor_tensor(
                out=stage_n[:].rearrange("p (a c) -> p a c", c=HC),
                in0=stage_n[:].rearrange("p (a c) -> p a c", c=HC),
                in1=bias_sb[:].rearrange("p (a c) -> p a c", a=1)
                    .broadcast_to([128, NPC, HC]),
                op=ALU.add)

            out_view = out_d[:].rearrange("(pr p) c -> p pr c", p=128)
            st_view = stage_n[:].rearrange("p (pr c) -> p pr c", c=HC)
            nc.sync.dma_start(out_view, st_view)

        for rep in range(repeat):
            emit_once(rep)
            if repeat > 1:
                tc.strict_bb_all_engine_barrier()

    nc.compile()
    return nc


def kernel(**inputs):
    cfg = _cfg()
    in_maps, meta = _prep(inputs, cfg)
    nc = _build_program(cfg, meta["K_LO"], meta["K_HI"])

    from concourse import bass_utils
    res = bass_utils.run_bass_kernel_spmd(
        nc, in_maps, core_ids=list(range(cfg["NC"])),
        trace=bool(int(os.environ.get("GAT_TRACE", "0"))),
    )
    kernel.last_result = res
    kernel.last_ctx = (nc, in_maps, cfg)

    NSHARD = cfg["NSHARD"]
    out_full = np.zeros((cfg["NC"] * NSHARD, HC), dtype=np.float32)
    for c in range(cfg["NC"]):
        out_full[c * NSHARD:(c + 1) * NSHARD] = res.results[c]["out"]
    return out_full[:cfg["N"]]
